# revision 23
# baseline (speedup 1.0000x reference)
"""BERT-CRF NER Viterbi decode kernel for Trainium2 (8 NeuronCores).

Strategy (data-parallel over batch, 8 rows/core), raw Bass (no Tile):
  - host: shard hidden_states [64,512,768] -> 8 x [8,512,768], pre-transpose to
    [8,768,512] so the PE matmul needs no on-device transpose; fold bias b into
    the transition matrix (feat enters the max additively per 'to').
  - device (per core):
      feats = W.T @ hsT per batch row -> PSUM [9,512] (6 K-chunks of 128)
      ACT copies PSUM->SBUF, DMA spreads to [(32*tc+b) partition, (to,tl)]
      transfeat[t,to,from] = trans[to,from]+b[to]+feat[t,to] (one bulk DVE op)
      Viterbi forward scan, t=1..511: 2 DVE ops per step on [8, 9x9]:
        scores = transfeat[t] + delta[t-1] (broadcast over 'to')
        delta[t] = reduce_max over 'from'   (stored for all t)
      bulk psi: argmax_from(trans[to,from]+delta[t-1,from]) for all t at once
        (is_ge/iota-encode/reduce trick; first-tie wins, matching jnp.argmax)
  - host: gather, backtrace (trivial pointer chase), return [64,512] int32.
"""

import numpy as np
from contextlib import ExitStack

import concourse.bass as bass
from concourse import mybir
from concourse.bass_utils import run_bass_kernel_spmd

B, T, H, L = 64, 512, 768, 9
NC = 8              # cores
BL = B // NC        # batch rows per core = 8
KC = H // 128       # 6 contraction chunks
TC = 4              # t-chunks of 128 for the spread layout
TL = T // TC        # 128
START = 7
NEG = -10000.0

F32 = mybir.dt.float32
ADD = mybir.AluOpType.add
MAX = mybir.AluOpType.max
GE = mybir.AluOpType.is_ge
MUL = mybir.AluOpType.mult
AXX = mybir.AxisListType.X


def build_program(debug=False):
    nc = bass.Bass("TRN2", target_bir_lowering=False,
                   detect_race_conditions=False)

    hsT_d = nc.dram_tensor("hsT", [BL, 128, KC * T], F32, kind="ExternalInput")
    wk_d = nc.dram_tensor("wk", [128, KC * L], F32, kind="ExternalInput")
    trep_d = nc.dram_tensor("trep", [128, L * L], F32, kind="ExternalInput")
    iot_d = nc.dram_tensor("iot", [128, L * L], F32, kind="ExternalInput")
    d0_d = nc.dram_tensor("d0", [BL, L], F32, kind="ExternalInput")
    psiv_d = nc.dram_tensor("psiv", [TC * BL, TL * L], F32, kind="ExternalOutput")
    dfin_d = nc.dram_tensor("dfin", [BL, L], F32, kind="ExternalOutput")
    if debug:
        dall_d = nc.dram_tensor("dall", [128, (TL + 1) * L], F32,
                                kind="ExternalOutput")
        tf_d = nc.dram_tensor("tfout", [128, TL * L * L], F32,
                              kind="ExternalOutput")
        stage_d = nc.dram_tensor("stageout", [L, BL * T], F32,
                                 kind="ExternalOutput")

    with ExitStack() as ctx:
        def sb(name, shape):
            return ctx.enter_context(nc.sbuf_tensor(name, shape, F32))
        wk = sb("wk_sb", [128, KC * L])
        trep = sb("trep_sb", [128, L * L])
        iot = sb("iot_sb", [128, L * L])
        # delta history, chunk-local: rows [32*tc .. 32*tc+8) slot j holds
        # delta_{128*tc + j - 1}; slot TL (=128) is the chunk's outgoing delta
        delta_all = sb("delta_all", [128, (TL + 1) * L])
        feats_sp = sb("feats_sp", [128, L * TL])
        mx = sb("mx", [128, TL * L])
        psiv = sb("psiv_sb", [128, TL * L])
        sc = sb("sc", [128, L * L])
        tf = sb("tf", [128, TL * L * L])
        sca = sb("sca", [128, TL * L * L])
        eq = sb("eq", [128, TL * L * L])
        ht = [sb(f"ht{i}", [128, KC * T]) for i in range(2)]  # double buffer
        stage = sb("stage", [L, BL * T])
        psum = [ctx.enter_context(nc.psum_tensor(f"psum{b}", [L, T], F32))
                for b in range(BL)]

        in_sem = ctx.enter_context(nc.semaphore("in_sem"))
        hs_sems = [ctx.enter_context(nc.semaphore(f"hs_sem{i}"))
                   for i in range(2)]
        pe_sem = ctx.enter_context(nc.semaphore("pe_sem"))
        cp_sem = ctx.enter_context(nc.semaphore("cp_sem"))
        sp_sem = ctx.enter_context(nc.semaphore("sp_sem"))
        ms_sem = ctx.enter_context(nc.semaphore("ms_sem"))
        dv_sem = ctx.enter_context(nc.semaphore("dv_sem"))
        bn_sem = ctx.enter_context(nc.semaphore("bn_sem"))
        bn2_sem = ctx.enter_context(nc.semaphore("bn2_sem"))
        out_sem = ctx.enter_context(nc.semaphore("out_sem"))
        block = ctx.enter_context(nc.Block())

        # trans (+bias) replicated, viewed [p, tl(bcast), to, from]
        def rep4(t_sb):
            return (t_sb[:, :].rearrange("p (a f) -> p a f", f=L)
                    .unsqueeze(1).broadcast_to([128, TL, L, L]))

        @block.gpsimd
        def _(g):
            # zero-fill (padding rows / slot 0 only matter for sim validity)
            g.memset(feats_sp[:, :], 0.0)
            g.memset(delta_all[:, :], 0.0).then_inc(ms_sem, 1)

        @block.sync
        def _(sync):
            sync.dma_start(wk[:, :], wk_d[:, :]).then_inc(in_sem, 16)
            sync.dma_start(trep[:, :], trep_d[:, :]).then_inc(in_sem, 16)
            sync.dma_start(iot[:, :], iot_d[:, :]).then_inc(in_sem, 16)
            for b in range(BL):
                if b >= 2:   # buffer free when b-2's matmuls done
                    sync.wait_ge(pe_sem, b - 1)
                sync.dma_start(ht[b % 2][:, :],
                               hsT_d[b, :, :]).then_inc(hs_sems[b % 2], 16)
            sync.wait_ge(ms_sem, 1)
            # delta_0 -> chunk 0 slot 1
            sync.dma_start(delta_all[0:BL, L:2 * L],
                           d0_d[:, :]).then_inc(in_sem, 16)
            # spread feats (stage -> feats_sp), per b after its ACT copy
            for b in range(BL):
                sync.wait_ge(cp_sem, b + 1)
                for t4 in range(TC):
                    d_ap = (feats_sp[t4 * 32 + b:t4 * 32 + b + 1, :]
                            .rearrange("p (to tl) -> p to tl", to=L))
                    s_ap = stage[:, b * T + t4 * TL:b * T + (t4 + 1) * TL]
                    sync.dma_start(d_ap, s_ap).then_inc(sp_sem, 16)
            # chunk-boundary delta copies
            for t4 in range(TC - 1):
                sync.wait_ge(bn_sem, t4 + 1)
                sync.dma_start(
                    delta_all[(t4 + 1) * 32:(t4 + 1) * 32 + BL, 0:L],
                    delta_all[t4 * 32:t4 * 32 + BL, TL * L:(TL + 1) * L],
                ).then_inc(bn2_sem, 16)
            # after scan: dfin out
            sync.wait_ge(dv_sem, 1)
            sync.dma_start(
                dfin_d[:, :],
                delta_all[96:96 + BL, TL * L:(TL + 1) * L],
            ).then_inc(out_sem, 16)
            if debug:
                sync.dma_start(dall_d[:, :],
                               delta_all[:, :]).then_inc(out_sem, 16)
                sync.dma_start(tf_d[:, :], tf[:, :]).then_inc(out_sem, 16)
                sync.dma_start(stage_d[:, :],
                               stage[:, :]).then_inc(out_sem, 16)
            # psiv out after bulk psi
            sync.wait_ge(dv_sem, 2)
            for t4 in range(TC):
                sync.dma_start(psiv_d[t4 * BL:(t4 + 1) * BL, :],
                               psiv[t4 * 32:t4 * 32 + BL, :]
                               ).then_inc(out_sem, 16)

        @block.tensor
        def _(te):
            te.wait_ge(in_sem, 48)                       # wk/trep/iot loaded
            for b in range(BL):
                te.wait_ge(hs_sems[b % 2], 16 * (b // 2 + 1))
                for kc in range(KC):
                    m = te.matmul(
                        psum[b][:, :],
                        wk[:, kc * L:(kc + 1) * L],
                        ht[b % 2][:, kc * T:(kc + 1) * T],
                        start=(kc == 0),
                        stop=(kc == KC - 1),
                    )
                    if kc == KC - 1:
                        m.then_inc(pe_sem, 1)

        @block.scalar
        def _(act):
            for b in range(BL):
                act.wait_ge(pe_sem, b + 1)
                act.copy(stage[:, b * T:(b + 1) * T],
                         psum[b][:, :]).then_inc(cp_sem, 1)

        @block.vector
        def _(v):
            # transfeat = trep + feats (bulk)
            v.wait_ge(sp_sem, 16 * TC * BL)
            v.wait_ge(in_sem, 64)                        # incl. d0
            in1 = (feats_sp[:, :].rearrange("p (to tl) -> p tl to", to=L)
                   .unsqueeze(3).broadcast_to([128, TL, L, L]))
            out4 = tf[:, :].rearrange("p (tl to f) -> p tl to f", to=L, f=L)
            v.tensor_tensor(out4, rep4(trep), in1, op=ADD)
            # Viterbi scan: step t reads chunk slot tl, writes slot tl+1
            for t in range(1, T):
                t4, tl = t // TL, t % TL
                base = t4 * 32
                if t4 > 0 and tl == 0:
                    v.wait_ge(bn2_sem, 16 * t4)          # boundary delta ready
                tf3 = (tf[base:base + BL, tl * L * L:(tl + 1) * L * L]
                       .rearrange("p (to f) -> p to f", to=L))
                d3 = (delta_all[base:base + BL, tl * L:(tl + 1) * L]
                      .rearrange("p (a f) -> p a f", a=1)
                      .broadcast_to([BL, L, L]))
                s3 = (sc[base:base + BL, :]
                      .rearrange("p (to f) -> p to f", to=L))
                v.tensor_tensor(s3, tf3, d3, op=ADD)
                r = v.tensor_reduce(
                    delta_all[base:base + BL, (tl + 1) * L:(tl + 2) * L],
                    s3, axis=AXX, op=MAX)
                if tl == TL - 1 and t4 < TC - 1:
                    r.then_inc(bn_sem, 1)                # chunk done
                # separate the reduce's tail write from the next TT's
                # head read (same-engine RAW on a pipelined engine)
                v.engine_nop()
            v.engine_nop().then_inc(dv_sem, 1)
            # bulk psi (delta_all slots 0..127 are exactly delta_{t-1})
            in1 = (delta_all[:, 0:TL * L].rearrange("p (tl f) -> p tl f", f=L)
                   .unsqueeze(2).broadcast_to([128, TL, L, L]))
            o4 = sca[:, :].rearrange("p (tl to f) -> p tl to f", to=L, f=L)
            v.tensor_tensor(o4, rep4(trep), in1, op=ADD)
            v.tensor_reduce(mx[:, :], o4, axis=AXX, op=MAX)
            e4 = eq[:, :].rearrange("p (tl to f) -> p tl to f", to=L, f=L)
            m4 = (mx[:, :].rearrange("p (tl to) -> p tl to", to=L)
                  .unsqueeze(3).broadcast_to([128, TL, L, L]))
            v.tensor_tensor(e4, o4, m4, op=GE)
            v.tensor_tensor(o4, e4, rep4(iot), op=MUL)
            v.tensor_reduce(psiv[:, :], o4, axis=AXX, op=MAX)
            v.engine_nop().then_inc(dv_sem, 1)

    return nc


_PROG = None


def _get_prog():
    global _PROG
    if _PROG is None:
        _PROG = build_program()
    return _PROG


def make_in_maps(hidden_states, W, b, transitions):
    hs = np.asarray(hidden_states, np.float32)
    W = np.asarray(W, np.float32)
    bb = np.asarray(b, np.float32)
    trans = np.asarray(transitions, np.float32)

    wk = np.ascontiguousarray(W.reshape(KC, 128, L).transpose(1, 0, 2)
                              ).reshape(128, KC * L)
    trep1 = (trans + bb[:, None]).reshape(1, L * L)
    trep = np.ascontiguousarray(np.broadcast_to(trep1, (128, L * L)))
    iota = np.broadcast_to((L - np.arange(L, dtype=np.float32))[None, :],
                           (L, L)).reshape(1, L * L)
    iot = np.ascontiguousarray(np.broadcast_to(iota, (128, L * L)))
    d0 = np.full((BL, L), NEG, np.float32)
    d0[:, START] = 0.0

    in_maps = []
    for c in range(NC):
        shard = hs[c * BL:(c + 1) * BL]                 # [8, 512, 768]
        # [8, 128(p), 6(kc), 512(t)]: row p holds hidden dims {kc*128+p}
        hsT = np.ascontiguousarray(
            shard.transpose(0, 2, 1).reshape(BL, KC, 128, T)
            .transpose(0, 2, 1, 3)).reshape(BL, 128, KC * T)
        in_maps.append({"hsT": hsT, "wk": wk, "trep": trep, "iot": iot,
                        "d0": d0})
    return in_maps


def decode_core(psiv, dfin):
    """psiv [32,1152] f32, dfin [8,9] f32 -> path [8,512] int32."""
    psi = (L - psiv.reshape(TC, BL, TL, L).transpose(1, 0, 2, 3)
           .reshape(BL, T, L)).astype(np.int32)          # [b, t, to], valid t>=1
    p = np.empty((BL, T), np.int32)
    p[:, T - 1] = np.argmax(dfin, axis=1)
    rows = np.arange(BL)
    for t in range(T - 1, 0, -1):
        p[:, t - 1] = psi[rows, t, p[:, t]]
    return p


def kernel(hidden_states, W, b, transitions):
    in_maps = make_in_maps(hidden_states, W, b, transitions)
    nc = _get_prog()
    res = run_bass_kernel_spmd(nc, in_maps, list(range(NC))).results
    path = np.empty((B, T), np.int32)
    for c in range(NC):
        path[c * BL:(c + 1) * BL] = decode_core(res[c]["psiv"], res[c]["dfin"])
    return path
